# revision 2
# baseline (speedup 1.0000x reference)
"""Contrastive pairwise-margin loss on 8 Trainium2 NeuronCores.

loss = sum_{i,j} [ R_ij * d_ij + (1-R_ij) * relu(0.5 - d_ij) ] / (N*(N-1)*2)
with d_ij = ||x_i - x_j||^2 and R_ij = [t_i == t_j].

Strategy:
- Host sorts rows by class (the double sum is permutation invariant), so all
  same-class pairs fall inside 512-wide diagonal blocks plus a 128x128 corner
  at each block boundary (requires max class size <= 128; checked, with an
  exact host-side fallback for any leftover pairs).
- Rewrite: loss_ij = relu(m - d_ij) + R_ij * (d_ij - relu(m - d_ij)).
  The first term is computed for ALL pairs; the R-masked second term only on
  the near-diagonal regions.
- The 16x16 grid of 512x512 blocks (upper triangle: 136 blocks) is split
  across 8 cores, 17 blocks each (stripes k and 15-k), off-diagonal blocks
  counted twice. Per [128,512] tile:
    * 3 accumulating matmuls (bf16) compute p = 2*x_i.x_j - sq_j into PSUM
      (sq_j folded in via 2 augmented K-rows: -sq_hi, -sq_lo).
    * one ScalarE activation computes relu(p + (0.5 - sq_i)) with per-
      partition bias and accumulates the per-row sum (accum_out).
    * diagonal blocks additionally compute R = onehot_i @ onehot_j^T on the
      TensorEngine and two fused VectorE ops accumulate sum(R*d), sum(R*a).
- Device returns per-tile partial sums [128, 68] + [128, 20]; host applies
  block weights and reduces in float64.
"""

import os
import sys

for _p in ("/opt/trn_rl_repo", "/root/.axon_site/_ro/trn_rl_repo"):
    if os.path.isdir(_p) and _p not in sys.path:
        sys.path.insert(0, _p)

from contextlib import ExitStack

import ml_dtypes
import numpy as np

import concourse.bass as bass  # noqa: F401
import concourse.mybir as mybir
from concourse import bacc, bass_utils
from concourse.tile import TileContext

BF16 = ml_dtypes.bfloat16
MARGIN = 0.5
N = 8192
D = 256
P = 128
BLK = 512          # block edge (rows/cols)
NBLK = N // BLK    # 16 stripes
NCORES = 8
NBLOCKS = 17       # blocks per core
NTILES = NBLOCKS * 4   # [128,512] tiles per core
NCORR = 20             # correction accum columns (2 ops x (8 diag tiles + 2 corners))

_COMPILED = None       # (nc, names) cache
LAST_RESULTS = None    # BassKernelResults of the last run (for profiling)


def _build_program():
    nc = bacc.Bacc("TRN2", target_bir_lowering=False, debug=False,
                   num_devices=NCORES)
    bf = mybir.dt.bfloat16
    f32 = mybir.dt.float32
    W = NBLOCKS * BLK  # 8704 packed columns

    la0 = nc.dram_tensor("la0", [P, W], bf, kind="ExternalInput")
    la1 = nc.dram_tensor("la1", [P, W], bf, kind="ExternalInput")
    la2 = nc.dram_tensor("la2", [2, W], bf, kind="ExternalInput")
    rb0 = nc.dram_tensor("rb0", [P, W], bf, kind="ExternalInput")
    rb1 = nc.dram_tensor("rb1", [P, W], bf, kind="ExternalInput")
    rb2 = nc.dram_tensor("rb2", [2, W], bf, kind="ExternalInput")
    oti = nc.dram_tensor("oti", [P, 2 * BLK], bf, kind="ExternalInput")
    otc = nc.dram_tensor("otc", [P, 2 * P], bf, kind="ExternalInput")
    bias_d = nc.dram_tensor("bias", [P, NTILES], f32, kind="ExternalInput")
    sqi_d = nc.dram_tensor("sqi", [P, 8], f32, kind="ExternalInput")
    acc_d = nc.dram_tensor("acc", [P, NTILES], f32, kind="ExternalOutput")
    cacc_d = nc.dram_tensor("cacc", [P, NCORR], f32, kind="ExternalOutput")

    Relu = mybir.ActivationFunctionType.Relu
    Alu = mybir.AluOpType

    with TileContext(nc) as tc, ExitStack() as ctx:
        sb = ctx.enter_context(tc.tile_pool(name="sb", bufs=1))
        apool = ctx.enter_context(tc.tile_pool(name="apool", bufs=4))
        rpool = ctx.enter_context(tc.tile_pool(name="rpool", bufs=2))
        scpool = ctx.enter_context(tc.tile_pool(name="scpool", bufs=2))
        pp = ctx.enter_context(tc.tile_pool(name="pp", bufs=4, space="PSUM"))
        rp = ctx.enter_context(tc.tile_pool(name="rp", bufs=2, space="PSUM"))

        la0_t = sb.tile([P, W], bf)
        la1_t = sb.tile([P, W], bf)
        la2_t = sb.tile([2, W], bf)
        rb0_t = sb.tile([P, W], bf)
        rb1_t = sb.tile([P, W], bf)
        rb2_t = sb.tile([2, W], bf)
        oti_t = sb.tile([P, 2 * BLK], bf)
        otc_t = sb.tile([P, 2 * P], bf)
        bias_t = sb.tile([P, NTILES], f32)
        sqi_t = sb.tile([P, 8], f32)
        acc_t = sb.tile([P, NTILES], f32)
        cacc_t = sb.tile([P, NCORR], f32)

        for t_, d_ in ((la0_t, la0), (la1_t, la1), (la2_t, la2),
                       (rb0_t, rb0), (rb1_t, rb1), (rb2_t, rb2),
                       (oti_t, oti), (otc_t, otc), (bias_t, bias_d),
                       (sqi_t, sqi_d)):
            nc.sync.dma_start(t_[:], d_[:])

        def corr_ops(p_ap, a_ap, r_sb, sq_col, out0, out1, w):
            # out0 += sum_j (p - sq_i)*R = -sum R*d ; out1 += sum_j a*R
            sc0 = scpool.tile([P, BLK], f32, tag="sc0")
            sc1 = scpool.tile([P, BLK], f32, tag="sc1")
            nc.vector.scalar_tensor_tensor(
                sc0[:, :w], p_ap, sqi_t[:, sq_col:sq_col + 1], r_sb,
                op0=Alu.subtract, op1=Alu.mult,
                accum_out=cacc_t[:, out0:out0 + 1])
            nc.vector.scalar_tensor_tensor(
                sc1[:, :w], a_ap, 0.0, r_sb,
                op0=Alu.add, op1=Alu.mult,
                accum_out=cacc_t[:, out1:out1 + 1])

        for t in range(NTILES):
            b, mi = divmod(t, 4)
            lo = b * BLK + mi * P
            p_t = pp.tile([P, BLK], mybir.dt.float32, tag="p")
            nc.tensor.matmul(p_t[:], la0_t[:, lo:lo + P],
                             rb0_t[:, b * BLK:(b + 1) * BLK],
                             start=True, stop=False)
            nc.tensor.matmul(p_t[:], la1_t[:, lo:lo + P],
                             rb1_t[:, b * BLK:(b + 1) * BLK],
                             start=False, stop=False)
            nc.tensor.matmul(p_t[:], la2_t[:2, lo:lo + P],
                             rb2_t[:2, b * BLK:(b + 1) * BLK],
                             start=False, stop=True)

            a_t = apool.tile([P, BLK], bf, tag="a")
            nc.scalar.activation(a_t[:], p_t[:], Relu,
                                 bias=bias_t[:, t:t + 1], scale=1.0,
                                 accum_out=acc_t[:, t:t + 1])

            if b < 2:
                # diagonal block: R over the whole [128, 512] tile
                r_ps = rp.tile([P, BLK], mybir.dt.float32, tag="r")
                nc.tensor.matmul(r_ps[:], oti_t[:, lo:lo + P],
                                 oti_t[:, b * BLK:(b + 1) * BLK],
                                 start=True, stop=True)
                r_sb = rpool.tile([P, BLK], bf, tag="rs")
                nc.vector.tensor_copy(r_sb[:], r_ps[:])
                ci = b * 4 + mi
                corr_ops(p_t[:], a_t[:], r_sb[:], ci, 2 * ci, 2 * ci + 1, BLK)
            elif b in (2, 3) and mi == 3:
                # corner: first 128 cols of the block, last m-tile rows
                c = b - 2
                r_ps = rp.tile([P, P], mybir.dt.float32, tag="rc")
                nc.tensor.matmul(r_ps[:], oti_t[:, c * BLK + 384:c * BLK + BLK],
                                 otc_t[:, c * P:(c + 1) * P],
                                 start=True, stop=True)
                r_sb = rpool.tile([P, P], bf, tag="rcs")
                nc.vector.tensor_copy(r_sb[:], r_ps[:])
                sq_col = c * 4 + 3
                corr_ops(p_t[:, 0:P], a_t[:, 0:P], r_sb[:],
                         sq_col, 16 + 2 * c, 17 + 2 * c, P)

        nc.sync.dma_start(acc_d[:], acc_t[:])
        nc.sync.dma_start(cacc_d[:], cacc_t[:])

    nc.compile()
    return nc


def _get_program():
    global _COMPILED
    if _COMPILED is None:
        _COMPILED = _build_program()
    return _COMPILED


def _core_blocks(k):
    """17 (row, col) blocks for core k; first two diagonal, next two carry
    the boundary corners (entry None = zero-padded corner)."""
    ra, rb = k, NBLK - 1 - k
    blocks_a = [(ra, c) for c in range(ra, NBLK)]
    blocks_b = [(rb, c) for c in range(rb, NBLK)]
    allb = set(blocks_a + blocks_b)
    diag = [(ra, ra), (rb, rb)]
    corn = [(ra, ra + 1)]
    corn_b = (rb, rb + 1)
    has_corn_b = corn_b in allb
    if has_corn_b:
        corn.append(corn_b)
    rest = sorted(allb - set(diag) - set(corn))
    if not has_corn_b:
        corn.append(rest.pop(0))  # filler block; its corner one-hot is zeroed
    order = diag + corn + rest
    assert len(order) == NBLOCKS
    return order, has_corn_b


def kernel(inputs: np.ndarray, target: np.ndarray) -> np.ndarray:
    global LAST_RESULTS
    x = np.asarray(inputs, dtype=np.float32)
    t = np.asarray(target).astype(np.int64)
    assert x.shape == (N, D) and t.shape == (N,)

    perm = np.argsort(t, kind="stable")
    xs = x[perm]
    ts = t[perm]

    sq64 = (xs.astype(np.float64) ** 2).sum(axis=1)
    sq = sq64.astype(np.float32)
    sq_hi = sq.astype(BF16)
    sq_lo = (sq - sq_hi.astype(np.float32)).astype(BF16)

    # augmented operands: p = sum_k lhs[k,i] * rhs[k,j] = 2*x_i.x_j - sq_j
    lhs0 = (2.0 * xs[:, :128]).astype(BF16).T.copy()      # [128, N]
    lhs1 = (2.0 * xs[:, 128:]).astype(BF16).T.copy()
    lhs2 = np.ones((2, N), dtype=BF16)
    rhs0 = xs[:, :128].astype(BF16).T.copy()
    rhs1 = xs[:, 128:].astype(BF16).T.copy()
    rhs2 = np.stack([-sq_hi, -sq_lo]).astype(BF16)        # [2, N]

    onehot = np.zeros((P, N), dtype=BF16)
    onehot[ts, np.arange(N)] = 1

    nclasses = int(ts.max()) + 1
    counts = np.bincount(ts, minlength=nclasses)
    leftover_pairs = counts.max() > P  # exact host fallback, ~never taken

    bias_all = (MARGIN - sq).astype(np.float32)

    in_maps = []
    weights = []
    for k in range(NCORES):
        order, has_corn_b = _core_blocks(k)
        W = NBLOCKS * BLK
        la0 = np.empty((P, W), BF16); la1 = np.empty((P, W), BF16)
        la2 = np.empty((2, W), BF16)
        rb0 = np.empty((P, W), BF16); rb1 = np.empty((P, W), BF16)
        rb2 = np.empty((2, W), BF16)
        bias = np.empty((P, NTILES), np.float32)
        for bidx, (r, c) in enumerate(order):
            rsl = slice(r * BLK, (r + 1) * BLK)
            csl = slice(c * BLK, (c + 1) * BLK)
            dst = slice(bidx * BLK, (bidx + 1) * BLK)
            la0[:, dst] = lhs0[:, rsl]; la1[:, dst] = lhs1[:, rsl]
            la2[:, dst] = lhs2[:, rsl]
            rb0[:, dst] = rhs0[:, csl]; rb1[:, dst] = rhs1[:, csl]
            rb2[:, dst] = rhs2[:, csl]
            for mi in range(4):
                rows = slice(r * BLK + mi * P, r * BLK + (mi + 1) * P)
                bias[:, bidx * 4 + mi] = bias_all[rows]
        ra, rbr = order[0][0], order[1][0]
        oti = np.concatenate([onehot[:, ra * BLK:(ra + 1) * BLK],
                              onehot[:, rbr * BLK:(rbr + 1) * BLK]], axis=1)
        otc = np.zeros((P, 2 * P), BF16)
        otc[:, 0:P] = onehot[:, (ra + 1) * BLK:(ra + 1) * BLK + P]
        if has_corn_b:
            otc[:, P:2 * P] = onehot[:, (rbr + 1) * BLK:(rbr + 1) * BLK + P]
        sqi = np.empty((P, 8), np.float32)
        for s, r in enumerate((ra, rbr)):
            for mi in range(4):
                rows = slice(r * BLK + mi * P, r * BLK + (mi + 1) * P)
                sqi[:, s * 4 + mi] = sq[rows]
        in_maps.append({"la0": la0, "la1": la1, "la2": la2,
                        "rb0": rb0, "rb1": rb1, "rb2": rb2,
                        "oti": oti, "otc": otc, "bias": bias, "sqi": sqi})
        weights.append(np.array([1.0 if (r == c) else 2.0
                                 for (r, c) in order]))

    nc = _get_program()
    res = bass_utils.run_bass_kernel_spmd(
        nc, in_maps, core_ids=list(range(NCORES)))
    LAST_RESULTS = res

    total = 0.0
    for k in range(NCORES):
        out = res.results[k]
        acc = out["acc"].astype(np.float64)    # [128, 68] sum of a per tile
        cacc = out["cacc"].astype(np.float64)  # [128, 20]
        w = np.repeat(weights[k], 4)           # per tile
        total += float((acc.sum(axis=0) * w).sum())
        # diagonal-block corrections (weight 1): sum R*d - sum R*a
        neg_rd = cacc[:, 0:16:2].sum()
        ra_ = cacc[:, 1:16:2].sum()
        total += (-neg_rd) - ra_
        # corner corrections (weight 2)
        neg_rd_c = cacc[:, 16::2].sum()
        ra_c = cacc[:, 17::2].sum()
        total += 2.0 * ((-neg_rd_c) - ra_c)

    if leftover_pairs:
        # exact fp64 host add for same-class pairs not covered by the
        # diag-block + corner regions (only if some class has > 128 rows)
        starts = np.concatenate([[0], np.cumsum(counts)])
        for c in range(nclasses):
            lo, hi = starts[c], starts[c + 1]
            if hi - lo <= P:
                continue
            idx = np.arange(lo, hi)
            ii, jj = np.meshgrid(idx, idx, indexing="ij")
            blk_i, blk_j = ii // BLK, jj // BLK
            covered = (blk_i == blk_j) | ((blk_j == blk_i + 1) &
                       (ii % BLK >= BLK - P) & (jj % BLK < P)) | \
                      ((blk_i == blk_j + 1) & (jj % BLK >= BLK - P) &
                       (ii % BLK < P))
            m = ~covered
            if m.any():
                xi = xs[ii[m]].astype(np.float64)
                xj = xs[jj[m]].astype(np.float64)
                dd = ((xi - xj) ** 2).sum(axis=1)
                total += float((dd - np.maximum(MARGIN - dd, 0.0)).sum())

    loss = total / (N * (N - 1.0) * 2.0)
    return np.float32(loss)


# revision 3
# speedup vs baseline: 1.4856x; 1.4856x over previous
"""Contrastive pairwise-margin loss on 8 Trainium2 NeuronCores.

loss = sum_{i,j} [ R_ij * d_ij + (1-R_ij) * relu(0.5 - d_ij) ] / (N*(N-1)*2)
with d_ij = ||x_i - x_j||^2 and R_ij = [t_i == t_j].

Strategy:
- Host sorts rows by class (the double sum is permutation invariant), so all
  same-class pairs fall inside 512-wide diagonal blocks plus a 128x128 corner
  at each block boundary (requires max class size <= 128; checked, with an
  exact host-side fallback for any leftover pairs).
- Rewrite: loss_ij = relu(m - d_ij) + R_ij * (d_ij - relu(m - d_ij)).
  The first term is computed for ALL pairs; the R-masked second term only on
  the near-diagonal regions.
- The 16x16 grid of 512x512 blocks (upper triangle: 136 blocks) is split
  across 8 cores, 17 blocks each (stripes k and 15-k), off-diagonal blocks
  counted twice. Per [128,512] tile:
    * 3 accumulating matmuls (bf16) compute p = 2*x_i.x_j - sq_j into PSUM
      (sq_j folded in via 2 augmented K-rows: -sq_hi, -sq_lo).
    * one ScalarE activation computes relu(p + (0.5 - sq_i)) with per-
      partition bias and accumulates the per-row sum (accum_out).
    * diagonal blocks additionally compute R = onehot_i @ onehot_j^T on the
      TensorEngine and two fused VectorE ops accumulate sum(R*d), sum(R*a).
- Device returns per-tile partial sums [128, 68] + [128, 20]; host applies
  block weights and reduces in float64.
"""

import os
import sys

for _p in ("/opt/trn_rl_repo", "/root/.axon_site/_ro/trn_rl_repo"):
    if os.path.isdir(_p) and _p not in sys.path:
        sys.path.insert(0, _p)

from contextlib import ExitStack

import ml_dtypes
import numpy as np

import concourse.bass as bass  # noqa: F401
import concourse.mybir as mybir
from concourse import bacc, bass_utils
from concourse.tile import TileContext

BF16 = ml_dtypes.bfloat16
MARGIN = 0.5
N = 8192
D = 256
P = 128
BLK = 512          # block edge (rows/cols)
NBLK = N // BLK    # 16 stripes
NCORES = 8
NBLOCKS = 17       # blocks per core
NTILES = NBLOCKS * 4   # [128,512] tiles per core
NCORR = 20             # correction accum columns (2 ops x (8 diag tiles + 2 corners))

_COMPILED = None       # (nc, names) cache
LAST_RESULTS = None    # BassKernelResults of the last run (for profiling)


def _build_program():
    nc = bacc.Bacc("TRN2", target_bir_lowering=False, debug=False,
                   num_devices=NCORES)
    bf = mybir.dt.bfloat16
    f32 = mybir.dt.float32
    W = NBLOCKS * BLK  # 8704 packed columns

    la0 = nc.dram_tensor("la0", [P, W], bf, kind="ExternalInput")
    la1 = nc.dram_tensor("la1", [P, W], bf, kind="ExternalInput")
    la2 = nc.dram_tensor("la2", [2, W], bf, kind="ExternalInput")
    rb0 = nc.dram_tensor("rb0", [P, W], bf, kind="ExternalInput")
    rb1 = nc.dram_tensor("rb1", [P, W], bf, kind="ExternalInput")
    rb2 = nc.dram_tensor("rb2", [2, W], bf, kind="ExternalInput")
    oti = nc.dram_tensor("oti", [P, 2 * BLK], bf, kind="ExternalInput")
    otc = nc.dram_tensor("otc", [P, 2 * P], bf, kind="ExternalInput")
    bias_d = nc.dram_tensor("bias", [P, NTILES], f32, kind="ExternalInput")
    sqi_d = nc.dram_tensor("sqi", [P, 8], f32, kind="ExternalInput")
    acc_d = nc.dram_tensor("acc", [P, NTILES], f32, kind="ExternalOutput")
    cacc_d = nc.dram_tensor("cacc", [P, NCORR], f32, kind="ExternalOutput")

    Relu = mybir.ActivationFunctionType.Relu
    Alu = mybir.AluOpType

    # tiles whose main relu+reduce runs on DVE instead of ACT (engine balance);
    # correction tiles (b<2) and corner tiles (t=11,15) stay on ACT since
    # their DVE budget is taken by the fused correction ops.
    free_tiles = [t for t in range(NTILES) if t >= 8 and t not in (11, 15)]
    DVE_TILES = frozenset(free_tiles[::2][:27])

    with TileContext(nc) as tc, ExitStack() as ctx:
        sb = ctx.enter_context(tc.tile_pool(name="sb", bufs=1))
        apool = ctx.enter_context(tc.tile_pool(name="apool", bufs=4))
        rpool = ctx.enter_context(tc.tile_pool(name="rpool", bufs=2))
        scpool = ctx.enter_context(tc.tile_pool(name="scpool", bufs=2))
        pp = ctx.enter_context(tc.tile_pool(name="pp", bufs=4, space="PSUM"))
        rp = ctx.enter_context(tc.tile_pool(name="rp", bufs=2, space="PSUM"))

        la2_t = sb.tile([2, W], bf)
        rb2_t = sb.tile([2, W], bf)
        oti_t = sb.tile([P, 2 * BLK], bf)
        otc_t = sb.tile([P, 2 * P], bf)
        bias_t = sb.tile([P, NTILES], f32)
        sqi_t = sb.tile([P, 8], f32)
        acc_t = sb.tile([P, NTILES], f32)
        cacc_t = sb.tile([P, NCORR], f32)

        # small tensors first so they never gate the pipeline
        for t_, d_ in ((bias_t, bias_d), (sqi_t, sqi_d), (oti_t, oti),
                       (otc_t, otc), (la2_t, la2), (rb2_t, rb2)):
            nc.sync.dma_start(t_[:], d_[:])

        # big operands: one tile + DMA per block so compute overlaps loading
        la0_b, la1_b, rb0_b, rb1_b = [], [], [], []
        for b in range(NBLOCKS):
            s = slice(b * BLK, (b + 1) * BLK)
            for lst, dram, nm in ((la0_b, la0, "la0"), (la1_b, la1, "la1"),
                                  (rb0_b, rb0, "rb0"), (rb1_b, rb1, "rb1")):
                t_ = sb.tile([P, BLK], bf, tag=f"{nm}b{b}")
                nc.sync.dma_start(t_[:], dram[:, s])
                lst.append(t_)

        def corr_ops(p_ap, a_ap, r_sb, sq_col, out0, out1, w):
            # out0 += sum_j (p - sq_i)*R = -sum R*d ; out1 += sum_j a*R
            sc0 = scpool.tile([P, BLK], f32, tag="sc0")
            sc1 = scpool.tile([P, BLK], f32, tag="sc1")
            nc.vector.scalar_tensor_tensor(
                sc0[:, :w], p_ap, sqi_t[:, sq_col:sq_col + 1], r_sb,
                op0=Alu.subtract, op1=Alu.mult,
                accum_out=cacc_t[:, out0:out0 + 1])
            nc.vector.scalar_tensor_tensor(
                sc1[:, :w], a_ap, 0.0, r_sb,
                op0=Alu.add, op1=Alu.mult,
                accum_out=cacc_t[:, out1:out1 + 1])

        for t in range(NTILES):
            b, mi = divmod(t, 4)
            lo = mi * P
            p_t = pp.tile([P, BLK], mybir.dt.float32, tag="p")
            nc.tensor.matmul(p_t[:], la0_b[b][:, lo:lo + P], rb0_b[b][:],
                             start=True, stop=False)
            nc.tensor.matmul(p_t[:], la1_b[b][:, lo:lo + P], rb1_b[b][:],
                             start=False, stop=False)
            glo = b * BLK + mi * P
            nc.tensor.matmul(p_t[:], la2_t[:2, glo:glo + P],
                             rb2_t[:2, b * BLK:(b + 1) * BLK],
                             start=False, stop=True)

            if t in DVE_TILES:
                a_t = apool.tile([P, BLK], bf, tag="adve")
                nc.vector.tensor_scalar(
                    a_t[:], p_t[:], bias_t[:, t:t + 1], 0.0,
                    op0=Alu.add, op1=Alu.max,
                    accum_out=acc_t[:, t:t + 1])
                continue

            a_t = apool.tile([P, BLK], bf, tag="a")
            nc.scalar.activation(a_t[:], p_t[:], Relu,
                                 bias=bias_t[:, t:t + 1], scale=1.0,
                                 accum_out=acc_t[:, t:t + 1])

            if b < 2:
                # diagonal block: R over the whole [128, 512] tile
                r_ps = rp.tile([P, BLK], mybir.dt.float32, tag="r")
                nc.tensor.matmul(r_ps[:], oti_t[:, b * BLK + lo:b * BLK + lo + P],
                                 oti_t[:, b * BLK:(b + 1) * BLK],
                                 start=True, stop=True)
                r_sb = rpool.tile([P, BLK], bf, tag="rs")
                nc.vector.tensor_copy(r_sb[:], r_ps[:])
                ci = b * 4 + mi
                corr_ops(p_t[:], a_t[:], r_sb[:], ci, 2 * ci, 2 * ci + 1, BLK)
            elif b in (2, 3) and mi == 3:
                # corner: first 128 cols of the block, last m-tile rows
                c = b - 2
                r_ps = rp.tile([P, P], mybir.dt.float32, tag="rc")
                nc.tensor.matmul(r_ps[:], oti_t[:, c * BLK + 384:c * BLK + BLK],
                                 otc_t[:, c * P:(c + 1) * P],
                                 start=True, stop=True)
                r_sb = rpool.tile([P, P], bf, tag="rcs")
                nc.vector.tensor_copy(r_sb[:], r_ps[:])
                sq_col = c * 4 + 3
                corr_ops(p_t[:, 0:P], a_t[:, 0:P], r_sb[:],
                         sq_col, 16 + 2 * c, 17 + 2 * c, P)

        nc.sync.dma_start(acc_d[:], acc_t[:])
        nc.sync.dma_start(cacc_d[:], cacc_t[:])

    nc.compile()
    return nc


def _get_program():
    global _COMPILED
    if _COMPILED is None:
        _COMPILED = _build_program()
    return _COMPILED


def _core_blocks(k):
    """17 (row, col) blocks for core k; first two diagonal, next two carry
    the boundary corners (entry None = zero-padded corner)."""
    ra, rb = k, NBLK - 1 - k
    blocks_a = [(ra, c) for c in range(ra, NBLK)]
    blocks_b = [(rb, c) for c in range(rb, NBLK)]
    allb = set(blocks_a + blocks_b)
    diag = [(ra, ra), (rb, rb)]
    corn = [(ra, ra + 1)]
    corn_b = (rb, rb + 1)
    has_corn_b = corn_b in allb
    if has_corn_b:
        corn.append(corn_b)
    rest = sorted(allb - set(diag) - set(corn))
    if not has_corn_b:
        corn.append(rest.pop(0))  # filler block; its corner one-hot is zeroed
    order = diag + corn + rest
    assert len(order) == NBLOCKS
    return order, has_corn_b


def kernel(inputs: np.ndarray, target: np.ndarray) -> np.ndarray:
    global LAST_RESULTS
    x = np.asarray(inputs, dtype=np.float32)
    t = np.asarray(target).astype(np.int64)
    assert x.shape == (N, D) and t.shape == (N,)

    perm = np.argsort(t, kind="stable")
    xs = x[perm]
    ts = t[perm]

    sq64 = (xs.astype(np.float64) ** 2).sum(axis=1)
    sq = sq64.astype(np.float32)
    sq_hi = sq.astype(BF16)
    sq_lo = (sq - sq_hi.astype(np.float32)).astype(BF16)

    # augmented operands: p = sum_k lhs[k,i] * rhs[k,j] = 2*x_i.x_j - sq_j
    lhs0 = (2.0 * xs[:, :128]).astype(BF16).T.copy()      # [128, N]
    lhs1 = (2.0 * xs[:, 128:]).astype(BF16).T.copy()
    lhs2 = np.ones((2, N), dtype=BF16)
    rhs0 = xs[:, :128].astype(BF16).T.copy()
    rhs1 = xs[:, 128:].astype(BF16).T.copy()
    rhs2 = np.stack([-sq_hi, -sq_lo]).astype(BF16)        # [2, N]

    onehot = np.zeros((P, N), dtype=BF16)
    onehot[ts, np.arange(N)] = 1

    nclasses = int(ts.max()) + 1
    counts = np.bincount(ts, minlength=nclasses)
    leftover_pairs = counts.max() > P  # exact host fallback, ~never taken

    bias_all = (MARGIN - sq).astype(np.float32)

    in_maps = []
    weights = []
    for k in range(NCORES):
        order, has_corn_b = _core_blocks(k)
        W = NBLOCKS * BLK
        la0 = np.empty((P, W), BF16); la1 = np.empty((P, W), BF16)
        la2 = np.empty((2, W), BF16)
        rb0 = np.empty((P, W), BF16); rb1 = np.empty((P, W), BF16)
        rb2 = np.empty((2, W), BF16)
        bias = np.empty((P, NTILES), np.float32)
        for bidx, (r, c) in enumerate(order):
            rsl = slice(r * BLK, (r + 1) * BLK)
            csl = slice(c * BLK, (c + 1) * BLK)
            dst = slice(bidx * BLK, (bidx + 1) * BLK)
            la0[:, dst] = lhs0[:, rsl]; la1[:, dst] = lhs1[:, rsl]
            la2[:, dst] = lhs2[:, rsl]
            rb0[:, dst] = rhs0[:, csl]; rb1[:, dst] = rhs1[:, csl]
            rb2[:, dst] = rhs2[:, csl]
            for mi in range(4):
                rows = slice(r * BLK + mi * P, r * BLK + (mi + 1) * P)
                bias[:, bidx * 4 + mi] = bias_all[rows]
        ra, rbr = order[0][0], order[1][0]
        oti = np.concatenate([onehot[:, ra * BLK:(ra + 1) * BLK],
                              onehot[:, rbr * BLK:(rbr + 1) * BLK]], axis=1)
        otc = np.zeros((P, 2 * P), BF16)
        otc[:, 0:P] = onehot[:, (ra + 1) * BLK:(ra + 1) * BLK + P]
        if has_corn_b:
            otc[:, P:2 * P] = onehot[:, (rbr + 1) * BLK:(rbr + 1) * BLK + P]
        sqi = np.empty((P, 8), np.float32)
        for s, r in enumerate((ra, rbr)):
            for mi in range(4):
                rows = slice(r * BLK + mi * P, r * BLK + (mi + 1) * P)
                sqi[:, s * 4 + mi] = sq[rows]
        in_maps.append({"la0": la0, "la1": la1, "la2": la2,
                        "rb0": rb0, "rb1": rb1, "rb2": rb2,
                        "oti": oti, "otc": otc, "bias": bias, "sqi": sqi})
        weights.append(np.array([1.0 if (r == c) else 2.0
                                 for (r, c) in order]))

    nc = _get_program()
    res = bass_utils.run_bass_kernel_spmd(
        nc, in_maps, core_ids=list(range(NCORES)))
    LAST_RESULTS = res

    total = 0.0
    for k in range(NCORES):
        out = res.results[k]
        acc = out["acc"].astype(np.float64)    # [128, 68] sum of a per tile
        cacc = out["cacc"].astype(np.float64)  # [128, 20]
        w = np.repeat(weights[k], 4)           # per tile
        total += float((acc.sum(axis=0) * w).sum())
        # diagonal-block corrections (weight 1): sum R*d - sum R*a
        neg_rd = cacc[:, 0:16:2].sum()
        ra_ = cacc[:, 1:16:2].sum()
        total += (-neg_rd) - ra_
        # corner corrections (weight 2)
        neg_rd_c = cacc[:, 16::2].sum()
        ra_c = cacc[:, 17::2].sum()
        total += 2.0 * ((-neg_rd_c) - ra_c)

    if leftover_pairs:
        # exact fp64 host add for same-class pairs not covered by the
        # diag-block + corner regions (only if some class has > 128 rows)
        starts = np.concatenate([[0], np.cumsum(counts)])
        for c in range(nclasses):
            lo, hi = starts[c], starts[c + 1]
            if hi - lo <= P:
                continue
            idx = np.arange(lo, hi)
            ii, jj = np.meshgrid(idx, idx, indexing="ij")
            blk_i, blk_j = ii // BLK, jj // BLK
            covered = (blk_i == blk_j) | ((blk_j == blk_i + 1) &
                       (ii % BLK >= BLK - P) & (jj % BLK < P)) | \
                      ((blk_i == blk_j + 1) & (jj % BLK >= BLK - P) &
                       (ii % BLK < P))
            m = ~covered
            if m.any():
                xi = xs[ii[m]].astype(np.float64)
                xj = xs[jj[m]].astype(np.float64)
                dd = ((xi - xj) ** 2).sum(axis=1)
                total += float((dd - np.maximum(MARGIN - dd, 0.0)).sum())

    loss = total / (N * (N - 1.0) * 2.0)
    return np.float32(loss)
